# revision 16
# baseline (speedup 1.0000x reference)
"""CIoU kernel for Trainium2: mean CIoU over 262144 pairs of convex CCW octagons.

Sort-free algorithm (validated against the jax reference):
  - intersection area via Green's theorem over mutually-clipped edges
    (each polygon edge clipped by the other polygon's 8 half-planes);
  - convex-hull(A union B) area via surviving-edge + bridge-edge tests using
    convex-cone (neighbor) conditions; one shared T-table serves both bridge
    directions (U_j == -T_j).
All quantities reduce to elementwise ops on pairwise cross-product tables,
vectorized item-per-lane across a [128 x T] layout, 8 cores data-parallel.

Engine discipline: every instruction reads tiles last written by at most ONE
other engine (TT sync-wait slots are very limited on this target).
"""
import sys

sys.path.insert(0, "/opt/trn_rl_repo")

import numpy as np
import concourse.bass as bass
import concourse.bacc as bacc
import concourse.tile as tile
from concourse import mybir
from concourse.bass_utils import run_bass_kernel_spmd

AOT = mybir.AluOpType
F32 = mybir.dt.float32
I32 = mybir.dt.int32

B = 262144
NCORES = 8
NI = B // NCORES          # items per core
P = 128                   # partitions
T = int(__import__("os").environ.get("T_OVERRIDE", 64))
CHUNK = P * T             # 8192 items per chunk
NCHUNK = int(__import__("os").environ.get("NCHUNK_OVERRIDE", NI // CHUNK))
EPS = 1e-12
BIG = 1e20


def build_program():
    nc = bacc.Bacc("TRN2", target_bir_lowering=False, debug=False, num_devices=NCORES)
    ab_d = nc.dram_tensor("ab", [NI, 32], F32, kind="ExternalInput")
    out_d = nc.dram_tensor("ciou", [NI], F32, kind="ExternalOutput")

    with tile.TileContext(nc) as tc:
        with tc.tile_pool(name="pool", bufs=1) as pool, \
             tc.tile_pool(name="spool", bufs=1) as spool:
            for ch in range(NCHUNK):
                _chunk(nc, pool, spool, ab_d, out_d, ch)
    nc.compile()
    return nc


def _chunk(nc, pool, spool, ab_d, out_d, ch):
    v = nc.vector
    g = nc.gpsimd
    s = nc.scalar
    BT = 64 * T           # dense [i,k,t] table free size
    dt = F32

    def big(tag, d=None):
        return pool.tile([P, BT], d or dt, tag=tag, name="b_" + tag)

    def small(tag):
        return spool.tile([P, 8 * T], dt, tag=tag, name="s_" + tag)

    def tiny(tag):
        return spool.tile([P, T], dt, tag=tag, name="t_" + tag)

    def dense(tl):  # [p, i, k, t]
        return tl.rearrange("p (i k t) -> p i k t", i=8, k=8)

    # ---------------- load raw coords (single DMA -> single wait) -----------
    raw = pool.tile([P, 32 * T], dt, tag="raw", name="raw")
    ab_view = ab_d[ch * CHUNK:(ch + 1) * CHUNK, :].rearrange(
        "(p t) jc -> p (t jc)", p=P)
    nc.gpsimd.dma_start(raw, ab_view)

    # coordinate views: [p, h, j, c, t]; f = t*32 + h*16 + j*2 + c
    rr = raw.rearrange("p (t h j c) -> p h j c t", h=2, j=8, c=2)
    ax = rr[:, 0, :, 0, :]; ay = rr[:, 0, :, 1, :]   # [p, 8, T]
    bx = rr[:, 1, :, 0, :]; by = rr[:, 1, :, 1, :]

    # ---------------- C table: C[j,k] = cross(a_j, b_k) ----------------
    ax_b = ax.unsqueeze(2).broadcast_to((P, 8, 8, T))
    ay_b = ay.unsqueeze(2).broadcast_to((P, 8, 8, T))
    bx_b = bx.unsqueeze(1).broadcast_to((P, 8, 8, T))
    by_b = by.unsqueeze(1).broadcast_to((P, 8, 8, T))
    t1 = big("sa")
    t2 = big("sb")
    v.tensor_tensor(dense(t1), ax_b, by_b, AOT.mult)
    g.tensor_tensor(dense(t2), ay_b, bx_b, AOT.mult)

    # padded C: 10x10 blocks, core at [1:9,1:9]; pads replicate cyclic wrap.
    # Core and pads all written by vector so every reader has one foreign dep
    # at most.
    cpad = pool.tile([P, 100 * T], dt, tag="cpad", name="cpad")
    cp = cpad.rearrange("p (si sk t) -> p si sk t", si=10, sk=10)
    C0 = cp[:, 1:9, 1:9, :]
    v.tensor_tensor(C0, dense(t1), dense(t2), AOT.subtract)
    v.tensor_copy(cp[:, 1:9, 0, :], cp[:, 1:9, 8, :])
    v.tensor_copy(cp[:, 1:9, 9, :], cp[:, 1:9, 1, :])
    v.tensor_copy(cp[:, 0, :, :], cp[:, 8, :, :])
    v.tensor_copy(cp[:, 9, :, :], cp[:, 1, :, :])

    # ---------------- adjacency cross products, padded 10 slots ----------------
    # adjX[i] = cross(x_i, x_{i+1}); slot layout s = i+1, s0 <- i=7, s9 <- i=0
    # adjA entirely on vector; adjB entirely on gpsimd.
    adjA = pool.tile([P, 10 * T], dt, tag="adja", name="adjA")
    adjB = pool.tile([P, 10 * T], dt, tag="adjb", name="adjB")
    apv = adjA.rearrange("p (s t) -> p s t", s=10)
    bpv = adjB.rearrange("p (s t) -> p s t", s=10)

    ua1 = spool.tile([P, 8 * T], dt, tag="adjt1", name="ua1")
    ua2 = spool.tile([P, 8 * T], dt, tag="adjt2", name="ua2")
    va1 = ua1.rearrange("p (s t) -> p s t", s=8)
    va2 = ua2.rearrange("p (s t) -> p s t", s=8)
    v.tensor_tensor(va1[:, 0:7, :], ax[:, 0:7, :], ay[:, 1:8, :], AOT.mult)
    v.tensor_tensor(va2[:, 0:7, :], ay[:, 0:7, :], ax[:, 1:8, :], AOT.mult)
    v.tensor_tensor(va1[:, 7, :], ax[:, 7, :], ay[:, 0, :], AOT.mult)
    v.tensor_tensor(va2[:, 7, :], ay[:, 7, :], ax[:, 0, :], AOT.mult)
    v.tensor_tensor(apv[:, 1:9, :], va1[:, :, :], va2[:, :, :], AOT.subtract)
    v.tensor_copy(apv[:, 0, :], apv[:, 8, :])
    v.tensor_copy(apv[:, 9, :], apv[:, 1, :])

    ub1 = spool.tile([P, 8 * T], dt, tag="adjt1", name="ub1")
    ub2 = spool.tile([P, 8 * T], dt, tag="adjt2", name="ub2")
    vb1 = ub1.rearrange("p (s t) -> p s t", s=8)
    vb2 = ub2.rearrange("p (s t) -> p s t", s=8)
    g.tensor_tensor(vb1[:, 0:7, :], bx[:, 0:7, :], by[:, 1:8, :], AOT.mult)
    g.tensor_tensor(vb2[:, 0:7, :], by[:, 0:7, :], bx[:, 1:8, :], AOT.mult)
    g.tensor_tensor(vb1[:, 7, :], bx[:, 7, :], by[:, 0, :], AOT.mult)
    g.tensor_tensor(vb2[:, 7, :], by[:, 7, :], bx[:, 0, :], AOT.mult)
    g.tensor_tensor(bpv[:, 1:9, :], vb1[:, :, :], vb2[:, :, :], AOT.subtract)
    g.tensor_copy(bpv[:, 0, :], bpv[:, 8, :])
    g.tensor_copy(bpv[:, 9, :], bpv[:, 1, :])

    adjA_c = apv[:, 1:9, :]      # [p,8,T] core
    adjB_c = bpv[:, 1:9, :]
    adjA_ik = adjA_c.unsqueeze(2).broadcast_to((P, 8, 8, T))   # bcast over k
    adjB_ik = adjB_c.unsqueeze(1).broadcast_to((P, 8, 8, T))   # bcast over i
    adjAm_ik = apv[:, 0:8, :].unsqueeze(2).broadcast_to((P, 8, 8, T))  # adjA[i-1]
    adjBm_ik = bpv[:, 0:8, :].unsqueeze(1).broadcast_to((P, 8, 8, T))  # adjB[k-1]

    Cip = cp[:, 2:10, 1:9, :]
    Ckp = cp[:, 1:9, 2:10, :]
    Cim = cp[:, 0:8, 1:9, :]
    Ckm = cp[:, 1:9, 0:8, :]
    Cipkp = cp[:, 2:10, 2:10, :]

    # ---------------- SA, SB, D ----------------
    # SA[i,k] = cross(eA_i, b_k - a_i) = Cip - C + adjA[i]
    # SB[i,k] = cross(eB_k, a_i - b_k) = C - Ckp + adjB[k]   (stored [i,k])
    # D[i,k]  = cross(eA_i, eB_k) = (Cipkp - Cip) + (C - Ckp)
    w1 = big("w1")
    v.tensor_tensor(dense(w1), Cip, C0, AOT.subtract)          # w1 = Cip - C
    SA = big("sa")
    g.tensor_tensor(dense(SA), dense(w1), adjA_ik, AOT.add)
    w2 = big("w2")
    v.tensor_tensor(dense(w2), C0, Ckp, AOT.subtract)          # w2 = C - Ckp
    SB = big("sb")
    g.tensor_tensor(dense(SB), dense(w2), adjB_ik, AOT.add)
    w3 = big("w3")
    v.tensor_tensor(dense(w3), Cipkp, Cip, AOT.subtract)       # w3 = Cipkp - Cip
    D = big("w1")
    v.tensor_tensor(dense(D), dense(w3), dense(w2), AOT.add)   # D = w3 + w2

    mpos = big("mpos", I32)
    v.tensor_scalar(dense(mpos), dense(D), 0.0, None, AOT.is_ge)
    mneg = big("mneg", I32)
    v.tensor_scalar(dense(mneg), dense(D), 0.0, None, AOT.is_lt)
    q1 = big("w2")
    v.tensor_scalar_max(dense(q1), dense(D), EPS)
    q2 = big("w3")
    v.tensor_scalar_min(dense(q2), dense(D), -EPS)
    v.copy_predicated(dense(q2), dense(mpos), dense(q1))       # q2 = Dsafe
    R = big("r")
    v.reciprocal(R, q2)

    # ---------------- A edges clipped by B half-planes (reduce over k) ----------
    ratioA = big("w1")
    g.tensor_tensor(dense(ratioA), dense(SB), dense(R), AOT.mult)
    mlo = big("w2")
    g.memset(mlo, -BIG)
    v.copy_predicated(dense(mlo), dense(mneg), dense(ratioA))
    loA = small("loa")
    lo_view = mlo.rearrange("p (i k t) -> p i t k", i=8, k=8)
    v.tensor_reduce(loA.rearrange("p (i t) -> p i t", i=8), lo_view,
                    axis=mybir.AxisListType.X, op=AOT.max)
    loA2 = small("loa2")
    v.tensor_scalar_max(loA2, loA, 0.0)
    mhi = big("w3")
    g.memset(mhi, BIG)
    v.copy_predicated(dense(mhi), dense(mpos), dense(ratioA))
    hiA = small("hia")
    hi_view = mhi.rearrange("p (i k t) -> p i t k", i=8, k=8)
    v.tensor_reduce(hiA.rearrange("p (i t) -> p i t", i=8), hi_view,
                    axis=mybir.AxisListType.X, op=AOT.min)
    hiA2 = small("hia2")
    v.tensor_scalar_min(hiA2, hiA, 1.0)
    wA = small("wa")
    v.tensor_tensor(wA, hiA2, loA2, AOT.subtract)
    wAr = small("war")
    v.tensor_scalar_max(wAr, wA, 0.0)
    wadjA = small("loa")
    v.tensor_tensor(wadjA.rearrange("p (i t) -> p i t", i=8),
                    wAr.rearrange("p (i t) -> p i t", i=8), adjA_c, AOT.mult)
    redA = tiny("reda")
    v.tensor_reduce(redA, wadjA.rearrange("p (i t) -> p t i", i=8),
                    axis=mybir.AxisListType.X, op=AOT.add)

    # ---------------- B edges clipped by A half-planes (reduce over i) ----------
    # sign-flipped form so the ratio is a plain Pool multiply:
    # rb' = SA*R = -ratioB_true;  wB = relu(min(m1,0) - max(x1,-1)) where
    # m1 = min over {D>=0} rb' (fill +BIG), x1 = max over {D<0} rb' (fill -BIG)
    ratioB = big("w1")
    g.tensor_tensor(dense(ratioB), dense(SA), dense(R), AOT.mult)
    mloB = big("w2")
    g.memset(mloB, BIG)
    v.copy_predicated(dense(mloB), dense(mpos), dense(ratioB))
    loB = small("lob")
    loB_view = mloB.rearrange("p (i k t) -> p k t i", i=8, k=8)
    v.tensor_reduce(loB.rearrange("p (k t) -> p k t", k=8), loB_view,
                    axis=mybir.AxisListType.X, op=AOT.min)
    loB2 = small("lob2")
    v.tensor_scalar_min(loB2, loB, 0.0)
    mhiB = big("w3")
    g.memset(mhiB, -BIG)
    v.copy_predicated(dense(mhiB), dense(mneg), dense(ratioB))
    hiB = small("hib")
    hiB_view = mhiB.rearrange("p (i k t) -> p k t i", i=8, k=8)
    v.tensor_reduce(hiB.rearrange("p (k t) -> p k t", k=8), hiB_view,
                    axis=mybir.AxisListType.X, op=AOT.max)
    hiB2 = small("hib2")
    v.tensor_scalar_max(hiB2, hiB, -1.0)
    wB = small("wb")
    v.tensor_tensor(wB, loB2, hiB2, AOT.subtract)
    wBr = small("wbr")
    v.tensor_scalar_max(wBr, wB, 0.0)
    wadjB = small("lob")
    g.tensor_tensor(wadjB.rearrange("p (k t) -> p k t", k=8),
                    wBr.rearrange("p (k t) -> p k t", k=8), adjB_c, AOT.mult)
    redB = tiny("redb")
    v.tensor_reduce(redB, wadjB.rearrange("p (k t) -> p t k", k=8),
                    axis=mybir.AxisListType.X, op=AOT.add)

    # ---------------- hull: surviving polygon edges ----------------
    minSA = small("hia")
    v.tensor_reduce(minSA.rearrange("p (i t) -> p i t", i=8),
                    SA.rearrange("p (i k t) -> p i t k", i=8, k=8),
                    axis=mybir.AxisListType.X, op=AOT.min)
    okEA = small("wa")
    v.tensor_scalar(okEA, minSA, 0.0, None, AOT.is_ge)
    eA = small("war")
    g.tensor_tensor(eA.rearrange("p (i t) -> p i t", i=8),
                    okEA.rearrange("p (i t) -> p i t", i=8), adjA_c, AOT.mult)
    redEA = tiny("redea")
    v.tensor_reduce(redEA, eA.rearrange("p (i t) -> p t i", i=8),
                    axis=mybir.AxisListType.X, op=AOT.add)
    minSB = small("hib")
    v.tensor_reduce(minSB.rearrange("p (k t) -> p k t", k=8),
                    SB.rearrange("p (i k t) -> p k t i", i=8, k=8),
                    axis=mybir.AxisListType.X, op=AOT.min)
    okEB = small("wb")
    v.tensor_scalar(okEB, minSB, 0.0, None, AOT.is_ge)
    eB = small("wbr")
    g.tensor_tensor(eB.rearrange("p (k t) -> p k t", k=8),
                    okEB.rearrange("p (k t) -> p k t", k=8), adjB_c, AOT.mult)
    redEB = tiny("redeb")
    v.tensor_reduce(redEB, eB.rearrange("p (k t) -> p t k", k=8),
                    axis=mybir.AxisListType.X, op=AOT.add)

    # ---------------- hull: bridges via shared T tables ----------------
    # T1 = C - Cim + adjA[i-1];  T2 = C - Cip - adjA[i]
    # T3 = C - Ckm - adjB[k-1];  T4 = C - Ckp + adjB[k]
    # bridge a_i->b_k iff min(T)>=0 ; bridge b_k->a_i iff max(T)<=0
    c1 = big("w2")
    v.tensor_tensor(dense(c1), C0, Cim, AOT.subtract)
    T1 = big("sa")
    g.tensor_tensor(dense(T1), dense(c1), adjAm_ik, AOT.add)
    c2 = big("w3")
    v.tensor_tensor(dense(c2), C0, Cip, AOT.subtract)
    T2 = big("sb")
    g.tensor_tensor(dense(T2), dense(c2), adjA_ik, AOT.subtract)
    tmin = big("w1")
    v.tensor_tensor(dense(tmin), dense(T1), dense(T2), AOT.min)
    tmax = big("w2")
    v.tensor_tensor(dense(tmax), dense(T1), dense(T2), AOT.max)
    c3 = big("w3")
    g.tensor_tensor(dense(c3), C0, Ckm, AOT.subtract)
    T3 = big("mpos")
    g.tensor_tensor(dense(T3), dense(c3), adjBm_ik, AOT.subtract)
    c4 = big("w3")
    g.tensor_tensor(dense(c4), C0, Ckp, AOT.subtract)
    T4 = big("mneg")
    g.tensor_tensor(dense(T4), dense(c4), adjB_ik, AOT.add)
    m34 = big("w3")
    v.tensor_tensor(dense(m34), dense(T3), dense(T4), AOT.min)
    x34 = big("sa")
    v.tensor_tensor(dense(x34), dense(T3), dense(T4), AOT.max)
    tminF = big("sb")
    v.tensor_tensor(dense(tminF), dense(tmin), dense(m34), AOT.min)
    tmaxF = big("w1")
    v.tensor_tensor(dense(tmaxF), dense(tmax), dense(x34), AOT.max)
    okAB = big("mpos")
    v.tensor_scalar(dense(okAB), dense(tminF), 0.0, None, AOT.is_ge)
    cAB = big("mneg")
    g.tensor_tensor(dense(cAB), dense(okAB), C0, AOT.mult)
    redAB = tiny("redab")
    v.tensor_reduce(redAB,
                    cAB.rearrange("p (i k t) -> p t i k", i=8, k=8),
                    axis=mybir.AxisListType.XY, op=AOT.add)
    okBA = big("w2")
    v.tensor_scalar(dense(okBA), dense(tmaxF), 0.0, None, AOT.is_le)
    cBA = big("w3")
    g.tensor_tensor(dense(cBA), dense(okBA), C0, AOT.mult)
    redBA = tiny("redba")
    v.tensor_reduce(redBA,
                    cBA.rearrange("p (i k t) -> p t i k", i=8, k=8),
                    axis=mybir.AxisListType.XY, op=AOT.add)

    # ---------------- per-item finals ----------------
    aA = tiny("aa")
    v.tensor_reduce(aA, adjA_c.rearrange("p i t -> p t i"),
                    axis=mybir.AxisListType.X, op=AOT.add)
    aB = tiny("ab")
    v.tensor_reduce(aB, adjB_c.rearrange("p i t -> p t i"),
                    axis=mybir.AxisListType.X, op=AOT.add)
    isum = tiny("isum")
    v.tensor_tensor(isum, redA, redB, AOT.add)
    inter = tiny("inter")
    s.mul(inter, isum, 0.5)
    asum = tiny("asum")
    v.tensor_tensor(asum, aA, aB, AOT.add)
    union = tiny("union")
    v.scalar_tensor_tensor(union, asum, 0.5, inter,
                           op0=AOT.mult, op1=AOT.subtract)
    h1 = tiny("h1")
    v.tensor_tensor(h1, redEA, redEB, AOT.add)
    h2 = tiny("h2")
    g.tensor_tensor(h2, h1, redAB, AOT.add)
    hsum = tiny("hsum")
    v.tensor_tensor(hsum, h2, redBA, AOT.subtract)
    rcu = tiny("rcu")
    v.reciprocal(rcu, union)
    rch = tiny("rch")
    v.reciprocal(rch, hsum)
    iou = tiny("iou")
    v.tensor_tensor(iou, inter, rcu, AOT.mult)
    ioum1 = tiny("ioum1")
    v.tensor_scalar_add(ioum1, iou, -1.0)
    qq = tiny("qq")
    g.tensor_tensor(qq, union, rch, AOT.mult)
    ciou = tiny("ciou")
    # ciou = 2*qq + (iou - 1)        [qq = union/hsum; union/ch = 2*union/hsum]
    v.scalar_tensor_tensor(ciou, qq, 2.0, ioum1,
                           op0=AOT.mult, op1=AOT.add)
    out_view = out_d[ch * CHUNK:(ch + 1) * CHUNK].rearrange("(p t) -> p t", p=P)
    nc.gpsimd.dma_start(out_view, ciou)


_CACHE = {}


def kernel(a: np.ndarray, b: np.ndarray) -> np.ndarray:
    a8 = a.reshape(NCORES, NI, 16)
    b8 = b.reshape(NCORES, NI, 16)
    ab = np.ascontiguousarray(np.concatenate([a8, b8], axis=2))
    if "nc" not in _CACHE:
        _CACHE["nc"] = build_program()
    nc = _CACHE["nc"]
    in_maps = [{"ab": ab[i]} for i in range(NCORES)]
    res = run_bass_kernel_spmd(nc, in_maps, list(range(NCORES)))
    total = np.float64(0.0)
    for i in range(NCORES):
        total += np.asarray(res.results[i]["ciou"], dtype=np.float64).sum()
    return np.float32(total / B)


# revision 17
# speedup vs baseline: 1.0322x; 1.0322x over previous
"""CIoU kernel for Trainium2: mean CIoU over 262144 pairs of convex CCW octagons.

Sort-free algorithm (validated against the jax reference):
  - intersection area via Green's theorem over mutually-clipped edges
    (each polygon edge clipped by the other polygon's 8 half-planes);
  - convex-hull(A union B) area via surviving-edge + bridge-edge tests using
    convex-cone (neighbor) conditions; one shared T-table serves both bridge
    directions (U_j == -T_j).
All quantities reduce to elementwise ops on pairwise cross-product tables,
vectorized item-per-lane across a [128 x T] layout, 8 cores data-parallel.

Engine discipline: every instruction reads tiles last written by at most ONE
other engine (TT sync-wait slots are very limited on this target).
"""
import sys

sys.path.insert(0, "/opt/trn_rl_repo")

import numpy as np
import concourse.bass as bass
import concourse.bacc as bacc
import concourse.tile as tile
from concourse import mybir
from concourse.bass_utils import run_bass_kernel_spmd

AOT = mybir.AluOpType
F32 = mybir.dt.float32
I32 = mybir.dt.int32

B = 262144
NCORES = 8
NI = B // NCORES          # items per core
P = 128                   # partitions
T = int(__import__("os").environ.get("T_OVERRIDE", 64))
CHUNK = P * T             # 8192 items per chunk
NCHUNK = int(__import__("os").environ.get("NCHUNK_OVERRIDE", NI // CHUNK))
EPS = 1e-12
BIG = 1e20


def build_program():
    nc = bacc.Bacc("TRN2", target_bir_lowering=False, debug=False, num_devices=NCORES)
    ab_d = nc.dram_tensor("ab", [NI, 32], F32, kind="ExternalInput")
    out_d = nc.dram_tensor("ciou", [NI], F32, kind="ExternalOutput")

    with tile.TileContext(nc) as tc:
        with tc.tile_pool(name="pool", bufs=1) as pool, \
             tc.tile_pool(name="spool", bufs=1) as spool:
            for ch in range(NCHUNK):
                _chunk(nc, pool, spool, ab_d, out_d, ch)
    nc.compile()
    return nc


def _chunk(nc, pool, spool, ab_d, out_d, ch):
    v = nc.vector
    g = nc.gpsimd
    s = nc.scalar
    BT = 64 * T           # dense [i,k,t] table free size
    dt = F32

    def big(tag, d=None):
        return pool.tile([P, BT], d or dt, tag=tag, name="b_" + tag)

    def small(tag):
        return spool.tile([P, 8 * T], dt, tag=tag, name="s_" + tag)

    def tiny(tag):
        return spool.tile([P, T], dt, tag=tag, name="t_" + tag)

    def dense(tl):  # [p, i, k, t]
        return tl.rearrange("p (i k t) -> p i k t", i=8, k=8)

    # ---------------- load raw coords (single DMA -> single wait) -----------
    raw = pool.tile([P, 32 * T], dt, tag="raw", name="raw")
    ab_view = ab_d[ch * CHUNK:(ch + 1) * CHUNK, :].rearrange(
        "(p t) jc -> p (t jc)", p=P)
    nc.gpsimd.dma_start(raw, ab_view)

    # coordinate views: [p, h, j, c, t]; f = t*32 + h*16 + j*2 + c
    rr = raw.rearrange("p (t h j c) -> p h j c t", h=2, j=8, c=2)
    ax = rr[:, 0, :, 0, :]; ay = rr[:, 0, :, 1, :]   # [p, 8, T]
    bx = rr[:, 1, :, 0, :]; by = rr[:, 1, :, 1, :]

    # ---------------- C table: C[j,k] = cross(a_j, b_k) ----------------
    ax_b = ax.unsqueeze(2).broadcast_to((P, 8, 8, T))
    ay_b = ay.unsqueeze(2).broadcast_to((P, 8, 8, T))
    bx_b = bx.unsqueeze(1).broadcast_to((P, 8, 8, T))
    by_b = by.unsqueeze(1).broadcast_to((P, 8, 8, T))
    t1 = big("sa")
    t2 = big("sb")
    v.tensor_tensor(dense(t1), ax_b, by_b, AOT.mult)
    g.tensor_tensor(dense(t2), ay_b, bx_b, AOT.mult)

    # padded C: 10x10 blocks, core at [1:9,1:9]; pads replicate cyclic wrap.
    # Core and pads all written by vector so every reader has one foreign dep
    # at most.
    cpad = pool.tile([P, 100 * T], dt, tag="cpad", name="cpad")
    cp = cpad.rearrange("p (si sk t) -> p si sk t", si=10, sk=10)
    C0 = cp[:, 1:9, 1:9, :]
    v.tensor_tensor(C0, dense(t1), dense(t2), AOT.subtract)
    v.tensor_copy(cp[:, 1:9, 0, :], cp[:, 1:9, 8, :])
    v.tensor_copy(cp[:, 1:9, 9, :], cp[:, 1:9, 1, :])
    v.tensor_copy(cp[:, 0, :, :], cp[:, 8, :, :])
    v.tensor_copy(cp[:, 9, :, :], cp[:, 1, :, :])

    # ---------------- adjacency cross products, padded 10 slots ----------------
    # adjX[i] = cross(x_i, x_{i+1}); slot layout s = i+1, s0 <- i=7, s9 <- i=0
    # adjA entirely on vector; adjB entirely on gpsimd.
    adjA = pool.tile([P, 10 * T], dt, tag="adja", name="adjA")
    adjB = pool.tile([P, 10 * T], dt, tag="adjb", name="adjB")
    apv = adjA.rearrange("p (s t) -> p s t", s=10)
    bpv = adjB.rearrange("p (s t) -> p s t", s=10)

    ua1 = spool.tile([P, 8 * T], dt, tag="adjt1", name="ua1")
    ua2 = spool.tile([P, 8 * T], dt, tag="adjt2", name="ua2")
    va1 = ua1.rearrange("p (s t) -> p s t", s=8)
    va2 = ua2.rearrange("p (s t) -> p s t", s=8)
    v.tensor_tensor(va1[:, 0:7, :], ax[:, 0:7, :], ay[:, 1:8, :], AOT.mult)
    v.tensor_tensor(va2[:, 0:7, :], ay[:, 0:7, :], ax[:, 1:8, :], AOT.mult)
    v.tensor_tensor(va1[:, 7, :], ax[:, 7, :], ay[:, 0, :], AOT.mult)
    v.tensor_tensor(va2[:, 7, :], ay[:, 7, :], ax[:, 0, :], AOT.mult)
    v.tensor_tensor(apv[:, 1:9, :], va1[:, :, :], va2[:, :, :], AOT.subtract)
    v.tensor_copy(apv[:, 0, :], apv[:, 8, :])
    v.tensor_copy(apv[:, 9, :], apv[:, 1, :])

    ub1 = spool.tile([P, 8 * T], dt, tag="adjt1", name="ub1")
    ub2 = spool.tile([P, 8 * T], dt, tag="adjt2", name="ub2")
    vb1 = ub1.rearrange("p (s t) -> p s t", s=8)
    vb2 = ub2.rearrange("p (s t) -> p s t", s=8)
    g.tensor_tensor(vb1[:, 0:7, :], bx[:, 0:7, :], by[:, 1:8, :], AOT.mult)
    g.tensor_tensor(vb2[:, 0:7, :], by[:, 0:7, :], bx[:, 1:8, :], AOT.mult)
    g.tensor_tensor(vb1[:, 7, :], bx[:, 7, :], by[:, 0, :], AOT.mult)
    g.tensor_tensor(vb2[:, 7, :], by[:, 7, :], bx[:, 0, :], AOT.mult)
    g.tensor_tensor(bpv[:, 1:9, :], vb1[:, :, :], vb2[:, :, :], AOT.subtract)
    g.tensor_copy(bpv[:, 0, :], bpv[:, 8, :])
    g.tensor_copy(bpv[:, 9, :], bpv[:, 1, :])

    adjA_c = apv[:, 1:9, :]      # [p,8,T] core
    adjB_c = bpv[:, 1:9, :]
    adjA_ik = adjA_c.unsqueeze(2).broadcast_to((P, 8, 8, T))   # bcast over k
    adjB_ik = adjB_c.unsqueeze(1).broadcast_to((P, 8, 8, T))   # bcast over i
    adjAm_ik = apv[:, 0:8, :].unsqueeze(2).broadcast_to((P, 8, 8, T))  # adjA[i-1]
    adjBm_ik = bpv[:, 0:8, :].unsqueeze(1).broadcast_to((P, 8, 8, T))  # adjB[k-1]

    Cip = cp[:, 2:10, 1:9, :]
    Ckp = cp[:, 1:9, 2:10, :]
    Cim = cp[:, 0:8, 1:9, :]
    Ckm = cp[:, 1:9, 0:8, :]
    Cipkp = cp[:, 2:10, 2:10, :]

    # ---------------- SA, SB, D ----------------
    # SA[i,k] = cross(eA_i, b_k - a_i) = Cip - C + adjA[i]
    # SB[i,k] = cross(eB_k, a_i - b_k) = C - Ckp + adjB[k]   (stored [i,k])
    # D[i,k]  = cross(eA_i, eB_k) = (Cipkp - Cip) + (C - Ckp)
    w1 = big("w1")
    v.tensor_tensor(dense(w1), Cip, C0, AOT.subtract)          # w1 = Cip - C
    SA = big("sa")
    g.tensor_tensor(dense(SA), dense(w1), adjA_ik, AOT.add)
    w2 = big("w2")
    v.tensor_tensor(dense(w2), C0, Ckp, AOT.subtract)          # w2 = C - Ckp
    SB = big("sb")
    g.tensor_tensor(dense(SB), dense(w2), adjB_ik, AOT.add)
    w3 = big("w3")
    v.tensor_tensor(dense(w3), Cipkp, Cip, AOT.subtract)       # w3 = Cipkp - Cip
    D = big("w1")
    v.tensor_tensor(dense(D), dense(w3), dense(w2), AOT.add)   # D = w3 + w2

    mpos = big("mpos", I32)
    v.tensor_scalar(dense(mpos), dense(D), 0.0, None, AOT.is_ge)
    mneg = big("mneg", I32)
    v.tensor_scalar(dense(mneg), dense(D), 0.0, None, AOT.is_lt)
    q1 = big("w2")
    v.tensor_scalar_max(dense(q1), dense(D), EPS)
    q2 = big("w3")
    v.tensor_scalar_min(dense(q2), dense(D), -EPS)
    v.copy_predicated(dense(q2), dense(mpos), dense(q1))       # q2 = Dsafe
    R = big("r")
    v.reciprocal(R, q2)

    # ---------------- A edges clipped by B half-planes (reduce over k) ----------
    ratioA = big("w1")
    g.tensor_tensor(dense(ratioA), dense(SB), dense(R), AOT.mult)
    mlo = big("w2")
    g.memset(mlo, -BIG)
    v.copy_predicated(dense(mlo), dense(mneg), dense(ratioA))
    loA = small("loa")
    lo_view = mlo.rearrange("p (i k t) -> p i t k", i=8, k=8)
    v.tensor_reduce(loA.rearrange("p (i t) -> p i t", i=8), lo_view,
                    axis=mybir.AxisListType.X, op=AOT.max)
    loA2 = small("loa2")
    v.tensor_scalar_max(loA2, loA, 0.0)
    mhi = big("w3")
    g.memset(mhi, BIG)
    v.copy_predicated(dense(mhi), dense(mpos), dense(ratioA))
    hiA = small("hia")
    hi_view = mhi.rearrange("p (i k t) -> p i t k", i=8, k=8)
    v.tensor_reduce(hiA.rearrange("p (i t) -> p i t", i=8), hi_view,
                    axis=mybir.AxisListType.X, op=AOT.min)
    hiA2 = small("hia2")
    v.tensor_scalar_min(hiA2, hiA, 1.0)
    wA = small("wa")
    v.tensor_tensor(wA, hiA2, loA2, AOT.subtract)
    wAr = small("war")
    v.tensor_scalar_max(wAr, wA, 0.0)
    wadjA = small("loa")
    v.tensor_tensor(wadjA.rearrange("p (i t) -> p i t", i=8),
                    wAr.rearrange("p (i t) -> p i t", i=8), adjA_c, AOT.mult)
    redA = tiny("reda")
    v.tensor_reduce(redA, wadjA.rearrange("p (i t) -> p t i", i=8),
                    axis=mybir.AxisListType.X, op=AOT.add)

    # ---------------- B edges clipped by A half-planes (reduce over i) ----------
    # sign-flipped form so the ratio is a plain Pool multiply:
    # rb' = SA*R = -ratioB_true;  wB = relu(min(m1,0) - max(x1,-1)) where
    # m1 = min over {D>=0} rb' (fill +BIG), x1 = max over {D<0} rb' (fill -BIG)
    ratioB = big("w1")
    g.tensor_tensor(dense(ratioB), dense(SA), dense(R), AOT.mult)
    mloB = big("w2")
    g.memset(mloB, BIG)
    v.copy_predicated(dense(mloB), dense(mpos), dense(ratioB))
    loB = small("lob")
    loB_view = mloB.rearrange("p (i k t) -> p k t i", i=8, k=8)
    v.tensor_reduce(loB.rearrange("p (k t) -> p k t", k=8), loB_view,
                    axis=mybir.AxisListType.X, op=AOT.min)
    loB2 = small("lob2")
    v.tensor_scalar_min(loB2, loB, 0.0)
    mhiB = big("w3")
    g.memset(mhiB, -BIG)
    v.copy_predicated(dense(mhiB), dense(mneg), dense(ratioB))
    hiB = small("hib")
    hiB_view = mhiB.rearrange("p (i k t) -> p k t i", i=8, k=8)
    v.tensor_reduce(hiB.rearrange("p (k t) -> p k t", k=8), hiB_view,
                    axis=mybir.AxisListType.X, op=AOT.max)
    hiB2 = small("hib2")
    v.tensor_scalar_max(hiB2, hiB, -1.0)
    wB = small("wb")
    v.tensor_tensor(wB, loB2, hiB2, AOT.subtract)
    wBr = small("wbr")
    v.tensor_scalar_max(wBr, wB, 0.0)
    wadjB = small("lob")
    g.tensor_tensor(wadjB.rearrange("p (k t) -> p k t", k=8),
                    wBr.rearrange("p (k t) -> p k t", k=8), adjB_c, AOT.mult)
    redB = tiny("redb")
    v.tensor_reduce(redB, wadjB.rearrange("p (k t) -> p t k", k=8),
                    axis=mybir.AxisListType.X, op=AOT.add)

    # ---------------- hull: surviving polygon edges ----------------
    minSA = small("hia")
    v.tensor_reduce(minSA.rearrange("p (i t) -> p i t", i=8),
                    SA.rearrange("p (i k t) -> p i t k", i=8, k=8),
                    axis=mybir.AxisListType.X, op=AOT.min)
    okEA = small("wa")
    v.tensor_scalar(okEA, minSA, 0.0, None, AOT.is_ge)
    eA = small("war")
    g.tensor_tensor(eA.rearrange("p (i t) -> p i t", i=8),
                    okEA.rearrange("p (i t) -> p i t", i=8), adjA_c, AOT.mult)
    redEA = tiny("redea")
    v.tensor_reduce(redEA, eA.rearrange("p (i t) -> p t i", i=8),
                    axis=mybir.AxisListType.X, op=AOT.add)
    minSB = small("hib")
    v.tensor_reduce(minSB.rearrange("p (k t) -> p k t", k=8),
                    SB.rearrange("p (i k t) -> p k t i", i=8, k=8),
                    axis=mybir.AxisListType.X, op=AOT.min)
    okEB = small("wb")
    v.tensor_scalar(okEB, minSB, 0.0, None, AOT.is_ge)
    eB = small("wbr")
    g.tensor_tensor(eB.rearrange("p (k t) -> p k t", k=8),
                    okEB.rearrange("p (k t) -> p k t", k=8), adjB_c, AOT.mult)
    redEB = tiny("redeb")
    v.tensor_reduce(redEB, eB.rearrange("p (k t) -> p t k", k=8),
                    axis=mybir.AxisListType.X, op=AOT.add)

    # ---------------- hull: bridges via shared T tables ----------------
    # T1 = C - Cim + adjA[i-1];  T2 = C - Cip - adjA[i]
    # T3 = C - Ckm - adjB[k-1];  T4 = C - Ckp + adjB[k]
    # bridge a_i->b_k iff min(T)>=0 ; bridge b_k->a_i iff max(T)<=0
    c1 = big("w2")
    v.tensor_tensor(dense(c1), C0, Cim, AOT.subtract)
    T1 = big("sa")
    g.tensor_tensor(dense(T1), dense(c1), adjAm_ik, AOT.add)
    c2 = big("w3")
    v.tensor_tensor(dense(c2), C0, Cip, AOT.subtract)
    T2 = big("sb")
    g.tensor_tensor(dense(T2), dense(c2), adjA_ik, AOT.subtract)
    tmin = big("w1")
    v.tensor_tensor(dense(tmin), dense(T1), dense(T2), AOT.min)
    tmax = big("w2")
    v.tensor_tensor(dense(tmax), dense(T1), dense(T2), AOT.max)
    c3 = big("w3")
    g.tensor_tensor(dense(c3), C0, Ckm, AOT.subtract)
    T3 = big("mpos")
    g.tensor_tensor(dense(T3), dense(c3), adjBm_ik, AOT.subtract)
    c4 = big("w3")
    g.tensor_tensor(dense(c4), C0, Ckp, AOT.subtract)
    T4 = big("mneg")
    g.tensor_tensor(dense(T4), dense(c4), adjB_ik, AOT.add)
    m34 = big("w3")
    v.tensor_tensor(dense(m34), dense(T3), dense(T4), AOT.min)
    x34 = big("sa")
    v.tensor_tensor(dense(x34), dense(T3), dense(T4), AOT.max)
    tminF = big("sb")
    v.tensor_tensor(dense(tminF), dense(tmin), dense(m34), AOT.min)
    tmaxF = big("w1")
    v.tensor_tensor(dense(tmaxF), dense(tmax), dense(x34), AOT.max)
    okAB = big("mpos")
    v.tensor_scalar(dense(okAB), dense(tminF), 0.0, None, AOT.is_ge)
    cAB = big("mneg")
    g.tensor_tensor(dense(cAB), dense(okAB), C0, AOT.mult)
    redAB = tiny("redab")
    v.tensor_reduce(redAB,
                    cAB.rearrange("p (i k t) -> p t i k", i=8, k=8),
                    axis=mybir.AxisListType.XY, op=AOT.add)
    okBA = big("w2")
    v.tensor_scalar(dense(okBA), dense(tmaxF), 0.0, None, AOT.is_le)
    cBA = big("w3")
    g.tensor_tensor(dense(cBA), dense(okBA), C0, AOT.mult)
    redBA = tiny("redba")
    v.tensor_reduce(redBA,
                    cBA.rearrange("p (i k t) -> p t i k", i=8, k=8),
                    axis=mybir.AxisListType.XY, op=AOT.add)

    # ---------------- per-item finals ----------------
    aA = tiny("aa")
    v.tensor_reduce(aA, adjA_c.rearrange("p i t -> p t i"),
                    axis=mybir.AxisListType.X, op=AOT.add)
    aB = tiny("ab")
    v.tensor_reduce(aB, adjB_c.rearrange("p i t -> p t i"),
                    axis=mybir.AxisListType.X, op=AOT.add)
    isum = tiny("isum")
    v.tensor_tensor(isum, redA, redB, AOT.add)
    inter = tiny("inter")
    s.mul(inter, isum, 0.5)
    asum = tiny("asum")
    v.tensor_tensor(asum, aA, aB, AOT.add)
    union = tiny("union")
    v.scalar_tensor_tensor(union, asum, 0.5, inter,
                           op0=AOT.mult, op1=AOT.subtract)
    h1 = tiny("h1")
    v.tensor_tensor(h1, redEA, redEB, AOT.add)
    h2 = tiny("h2")
    g.tensor_tensor(h2, h1, redAB, AOT.add)
    hsum = tiny("hsum")
    v.tensor_tensor(hsum, h2, redBA, AOT.subtract)
    rcu = tiny("rcu")
    v.reciprocal(rcu, union)
    rch = tiny("rch")
    v.reciprocal(rch, hsum)
    iou = tiny("iou")
    v.tensor_tensor(iou, inter, rcu, AOT.mult)
    ioum1 = tiny("ioum1")
    v.tensor_scalar_add(ioum1, iou, -1.0)
    qq = tiny("qq")
    g.tensor_tensor(qq, union, rch, AOT.mult)
    ciou = tiny("ciou")
    # ciou = 2*qq + (iou - 1)        [qq = union/hsum; union/ch = 2*union/hsum]
    v.scalar_tensor_tensor(ciou, qq, 2.0, ioum1,
                           op0=AOT.mult, op1=AOT.add)
    out_view = out_d[ch * CHUNK:(ch + 1) * CHUNK].rearrange("(p t) -> p t", p=P)
    nc.gpsimd.dma_start(out_view, ciou)


_CACHE = {}


def _get_executable():
    if "exec" in _CACHE:
        return _CACHE["exec"]
    import jax
    from jax.sharding import Mesh, PartitionSpec, NamedSharding
    from jax.experimental.shard_map import shard_map
    from concourse import bass2jax

    nc = build_program()
    bass2jax.install_neuronx_cc_hook()

    partition_name = nc.partition_id_tensor.name if nc.partition_id_tensor else None
    in_names, out_names, out_avals = [], [], []
    for alloc in nc.m.functions[0].allocations:
        if not isinstance(alloc, mybir.MemoryLocationSet):
            continue
        name = alloc.memorylocations[0].name
        if alloc.kind == "ExternalInput":
            if name != partition_name:
                in_names.append(name)
        elif alloc.kind == "ExternalOutput":
            out_names.append(name)
            out_avals.append(jax.core.ShapedArray(
                tuple(alloc.tensor_shape), mybir.dt.np(alloc.dtype)))
    all_names = in_names + out_names
    if partition_name is not None:
        all_names = all_names + [partition_name]

    def _body(*args):
        operands = list(args)
        if partition_name is not None:
            operands.append(bass2jax.partition_id_tensor())
        outs = bass2jax._bass_exec_p.bind(
            *operands,
            out_avals=tuple(out_avals),
            in_names=tuple(all_names),
            out_names=tuple(out_names),
            lowering_input_output_aliases=(),
            sim_require_finite=True,
            sim_require_nnan=True,
            nc=nc,
        )
        return tuple(outs)

    devices = jax.devices()[:NCORES]
    mesh = Mesh(np.asarray(devices), ("core",))
    nin = len(in_names)
    nout = len(out_names)
    sharded = jax.jit(
        shard_map(_body, mesh=mesh,
                  in_specs=(PartitionSpec("core"),) * (nin + nout),
                  out_specs=(PartitionSpec("core"),) * nout,
                  check_rep=False),
        keep_unused=True)
    sh = NamedSharding(mesh, PartitionSpec("core"))
    zeros = [np.zeros((NCORES * av.shape[0], *av.shape[1:]), av.dtype)
             for av in out_avals]
    _CACHE["exec"] = (sharded, sh, zeros)
    return _CACHE["exec"]


def kernel(a: np.ndarray, b: np.ndarray) -> np.ndarray:
    import jax
    a8 = np.asarray(a, dtype=np.float32).reshape(NCORES, NI, 16)
    b8 = np.asarray(b, dtype=np.float32).reshape(NCORES, NI, 16)
    ab = np.ascontiguousarray(np.concatenate([a8, b8], axis=2))
    sharded, sh, zeros = _get_executable()
    ab_dev = jax.device_put(ab.reshape(NCORES * NI, 32), sh)
    zeros_dev = [jax.device_put(z, sh) for z in zeros]
    out = sharded(ab_dev, *zeros_dev)
    ciou = np.asarray(out[0], dtype=np.float64)
    return np.float32(ciou.sum() / B)
